# revision 14
# baseline (speedup 1.0000x reference)
"""Trainium2 Bass kernel for nn_BLoraLinear (batched multi-adapter LoRA linear).

Math:  out = x @ W.T + b + sum_s sum_m mask_s(t) * (x @ A[m,s]) @ B[m,s]

Reformulation (exact): with per-(module,segment) adapter columns packed
into Ahat [D_IN, r_hat] / Bhat [r_hat, D_OUT] and a per-token segment
mask MT [r_hat, T],
    out = x @ W.T + b + ((x @ Ahat) * MT.T) @ Bhat
which fuses into one K=(D_IN + r_hat) contraction per output tile:
    out = [x, u] @ [W.T ; Bhat] + b,   u = (x @ Ahat) * MT.T

Sharding: data-parallel over tokens, 1024 tokens per core, zero
collectives.  Since the host knows cu_seqlen values, each core packs
only the adapters of segments overlapping its token range (slots).  Up
to 4 active segments -> r_hat=128 (one contraction chunk); rare draws
with more fall back to a precompiled r_hat=256 variant (always exact).

All matmul operands pre-cast to bf16 on host; f32 accumulation in PSUM;
bias is added during PSUM eviction on the vector engine.
"""

import numpy as np
import ml_dtypes

# Problem shape (hardcoded per spec nn_BLoraLinear_46471546143180).
T, D_IN, D_OUT, R, M, S = 8192, 4096, 4096, 16, 2, 8
N_CORES = 8
T_C = T // N_CORES
MR = M * R                    # adapter columns per segment (32)

BF16 = ml_dtypes.bfloat16


def _build(t_c, d_in, d_out, r_hat):
    """Per-core Bass/Tile program (same NEFF on all cores).

    DRAM layouts are host-prearranged so every DMA is contiguous per
    partition:
      xt   [128, KX, t_c]       xt[p,a,t]    = x[tok0+t, a*128+p]      bf16
      wt   [NB, 128, KX, 512]   wt[n,p,a,c]  = W.T[a*128+p, n*512+c]   bf16
      bh   [128, RC, NB, 512]   bh[p,r,n,c]  = Bhat[r*128+p, n*512+c]  bf16
      ah   [128, KX, r_hat]     ah[p,a,j]    = Ahat[a*128+p, j]        bf16
      mt   [128, RC, t_c]       mt[p,r,t]    = MT[r*128+p, tok0+t]     bf16
      brep [128, d_out]         bias replicated across partitions      bf16
      out  [t_c, d_out]                                                f32
    """
    import concourse.bacc as bacc
    import concourse.mybir as mybir
    from concourse.tile import TileContext

    dt = mybir.dt
    KX = d_in // 128
    RC = r_hat // 128
    NB = d_out // 512
    MB = t_c // 128
    TB = t_c // 512

    nc = bacc.Bacc("TRN2", target_bir_lowering=False)

    xt = nc.dram_tensor("xt", [128, KX, t_c], dt.bfloat16, kind="ExternalInput")
    wt = nc.dram_tensor("wt", [NB, 128, KX, 512], dt.bfloat16, kind="ExternalInput")
    bh = nc.dram_tensor("bh", [128, RC, NB, 512], dt.bfloat16, kind="ExternalInput")
    ah = nc.dram_tensor("ah", [128, KX, r_hat], dt.bfloat16, kind="ExternalInput")
    mt = nc.dram_tensor("mt", [128, RC, t_c], dt.bfloat16, kind="ExternalInput")
    brep = nc.dram_tensor("brep", [128, d_out], dt.bfloat16, kind="ExternalInput")
    out = nc.dram_tensor("out", [t_c, d_out], dt.float32, kind="ExternalOutput")
    warm = nc.dram_tensor("warm", [128, 512], dt.float32, kind="ExternalOutput")

    with TileContext(nc) as tc:
        with tc.tile_pool(name="resident", bufs=1) as res_pool, \
             tc.tile_pool(name="wpool", bufs=2) as w_pool, \
             tc.tile_pool(name="ps", bufs=8, space="PSUM") as ps_pool, \
             tc.tile_pool(name="opool", bufs=4) as o_pool:
            xt_sb = res_pool.tile([128, KX, t_c], dt.bfloat16, name="xt_sb")
            ah_sb = res_pool.tile([128, KX, r_hat], dt.bfloat16, name="ah_sb")
            bh_sb = res_pool.tile([128, RC, NB, 512], dt.bfloat16, name="bh_sb")
            mt_sb = res_pool.tile([128, RC, t_c], dt.bfloat16, name="mt_sb")
            ut_sb = res_pool.tile([128, RC, t_c], dt.bfloat16, name="ut_sb")
            brep_sb = res_pool.tile([128, d_out], dt.bfloat16, name="brep_sb")

            # HAM warmup: PE sits idle ~12us waiting for input DMAs, which
            # leaves the clock gate at 1.2 GHz for the first ~3.4us of real
            # matmuls.  Burn that idle time on dummy matmuls instead so the
            # real stream starts at full clock.  Chained to a scratch
            # output so the instructions aren't dead.
            warm_sb = res_pool.tile([128, 512], dt.bfloat16, name="warm_sb")
            nc.vector.memset(warm_sb[:], 0.0)
            warm_ps = ps_pool.tile([128, 512], dt.float32, name="warm_ps",
                                   tag="ps")
            NWARM = 32
            for i in range(NWARM):
                nc.tensor.matmul(
                    warm_ps[:], warm_sb[:, 0:128], warm_sb[:],
                    start=(i == 0), stop=(i == NWARM - 1))
            warm_o = res_pool.tile([128, 512], dt.float32, name="warm_o")
            nc.vector.tensor_copy(out=warm_o[:], in_=warm_ps[:])
            nc.sync.dma_start(out=warm[:], in_=warm_o[:])

            # Load order tracks first use: ah + the first token-half of x
            # (phase A tb=0 and phase B m<MB/2 only touch that half), then
            # the mask, then W.T block 0 in k-chunk pieces, then the rest.
            step = max(1, KX // 8)
            for a0 in range(0, KX, step):
                a1 = min(a0 + step, KX)
                nc.sync.dma_start(out=ah_sb[:, a0:a1, :], in_=ah[:, a0:a1, :])
                nc.sync.dma_start(out=xt_sb[:, a0:a1, 0:t_c // 2],
                                  in_=xt[:, a0:a1, 0:t_c // 2])
            nc.sync.dma_start(out=mt_sb[:], in_=mt[:])

            # prefetch W.T column-block n=0 in k-chunk pieces so phase-B
            # matmuls can begin before the whole 4 MB block lands
            wn_tiles = {}

            def load_wn(n):
                t = w_pool.tile([128, KX, 512], dt.bfloat16, name="wn", tag="wn")
                for a0 in range(0, KX, step):
                    a1 = min(a0 + step, KX)
                    nc.sync.dma_start(out=t[:, a0:a1, :], in_=wt[n, :, a0:a1, :])
                wn_tiles[n] = t

            load_wn(0)
            nc.sync.dma_start(out=bh_sb[:], in_=bh[:])
            for a0 in range(0, KX, step):
                a1 = min(a0 + step, KX)
                nc.sync.dma_start(out=xt_sb[:, a0:a1, t_c // 2:],
                                  in_=xt[:, a0:a1, t_c // 2:])
            nc.sync.dma_start(out=brep_sb[:], in_=brep[:])

            # Phase A: uT[j, t] = mask[j, t] * sum_k Ahat[k, j] * xT[k, t]
            for tb in range(TB):
                for rc in range(RC):
                    ps_u = ps_pool.tile([128, 512], dt.float32, name="ps_u", tag="ps")
                    for k in range(KX):
                        nc.tensor.matmul(
                            ps_u[:],
                            ah_sb[:, k, rc * 128:(rc + 1) * 128],
                            xt_sb[:, k, tb * 512:(tb + 1) * 512],
                            start=(k == 0),
                            stop=(k == KX - 1),
                        )
                    nc.vector.tensor_mul(
                        out=ut_sb[:, rc, tb * 512:(tb + 1) * 512],
                        in0=ps_u[:],
                        in1=mt_sb[:, rc, tb * 512:(tb + 1) * 512],
                    )

            # Phase B: out[t, d] = b[d] + sum_k xT/uT[k, t] * [W.T;Bhat][k, d]
            for n in range(NB):
                if n + 1 < NB:
                    load_wn(n + 1)
                wn = wn_tiles.pop(n)
                for m in range(MB):
                    ps_o = ps_pool.tile([128, 512], dt.float32, name="ps_o", tag="ps")
                    for k in range(KX):
                        nc.tensor.matmul(
                            ps_o[:],
                            xt_sb[:, k, m * 128:(m + 1) * 128],
                            wn[:, k, :],
                            start=(k == 0), stop=False,
                        )
                    for r in range(RC):
                        nc.tensor.matmul(
                            ps_o[:],
                            ut_sb[:, r, m * 128:(m + 1) * 128],
                            bh_sb[:, r, n, :],
                            start=False, stop=(r == RC - 1),
                        )
                    o_sb = o_pool.tile([128, 512], dt.float32, name="o_sb")
                    nc.vector.tensor_add(
                        out=o_sb[:], in0=ps_o[:],
                        in1=brep_sb[:, n * 512:(n + 1) * 512])
                    nc.sync.dma_start(
                        out=out[m * 128:(m + 1) * 128, n * 512:(n + 1) * 512],
                        in_=o_sb[:],
                    )

    nc.compile()
    nc.finalize()
    return nc


def _core_slots(cu, t_c, n_cores, n_slots):
    """Per-core list of segments overlapping the core's token range,
    padded with -1 to n_slots.  Returns None if any core needs more."""
    out = []
    for c in range(n_cores):
        lo, hi = c * t_c, (c + 1) * t_c
        slots = [s for s in range(S) if cu[s] < hi and cu[s + 1] > lo
                 and cu[s + 1] > cu[s]]
        if len(slots) > n_slots:
            return None
        out.append(slots + [-1] * (n_slots - len(slots)))
    return out


def _prep_in_maps(x, W, b, lora_A, lora_B, cu_seqlen):
    x = np.asarray(x, dtype=np.float32)
    W = np.asarray(W, dtype=np.float32)
    b = np.asarray(b, dtype=np.float32)
    lora_A = np.asarray(lora_A, dtype=np.float32)
    lora_B = np.asarray(lora_B, dtype=np.float32)
    cu = np.asarray(cu_seqlen).astype(np.int64)

    # full Ahat[k, j], Bhat[j, d], j = (s*M + m)*R + r
    Ahat = np.transpose(lora_A, (2, 1, 0, 3)).reshape(D_IN, S * MR).astype(BF16)
    Bhat = np.transpose(lora_B, (1, 0, 2, 3)).reshape(S * MR, D_OUT).astype(BF16)

    r_hat = 128
    slots = _core_slots(cu, T_C, N_CORES, r_hat // MR)
    if slots is None:
        r_hat = S * MR                                   # 256 fallback
        slots = [list(range(S)) for _ in range(N_CORES)]

    KX = D_IN // 128
    RC = r_hat // 128
    NB = D_OUT // 512

    wt_host = np.ascontiguousarray(
        W.T.astype(BF16).reshape(KX, 128, NB, 512).transpose(2, 1, 0, 3))
    brep_host = np.ascontiguousarray(
        np.broadcast_to(b.astype(BF16), (128, D_OUT)))

    xT = x.astype(BF16).T                                # [D_IN, T] view
    tok = np.arange(T_C)
    in_maps = []
    for c in range(N_CORES):
        sl = slice(c * T_C, (c + 1) * T_C)
        xt_host = np.ascontiguousarray(
            xT[:, sl].reshape(KX, 128, T_C).transpose(1, 0, 2))

        Ah_c = np.zeros((D_IN, r_hat), dtype=BF16)
        Bh_c = np.zeros((r_hat, D_OUT), dtype=BF16)
        MT_c = np.zeros((r_hat, T_C), dtype=BF16)
        for a, s in enumerate(slots[c]):
            if s < 0:
                continue
            Ah_c[:, a * MR:(a + 1) * MR] = Ahat[:, s * MR:(s + 1) * MR]
            Bh_c[a * MR:(a + 1) * MR, :] = Bhat[s * MR:(s + 1) * MR, :]
            lo = max(int(cu[s]) - c * T_C, 0)
            hi = min(int(cu[s + 1]) - c * T_C, T_C)
            if hi > lo:
                MT_c[a * MR:(a + 1) * MR, lo:hi] = 1.0

        ah_host = np.ascontiguousarray(
            Ah_c.reshape(KX, 128, r_hat).transpose(1, 0, 2))
        bh_host = np.ascontiguousarray(
            Bh_c.reshape(RC, 128, NB, 512).transpose(1, 0, 2, 3))
        mt_host = np.ascontiguousarray(
            MT_c.reshape(RC, 128, T_C).transpose(1, 0, 2))
        in_maps.append({
            "xt": xt_host, "wt": wt_host, "bh": bh_host, "ah": ah_host,
            "mt": mt_host, "brep": brep_host,
        })
    return in_maps, r_hat


_NC_CACHE = {}


def _get_nc(r_hat):
    key = (T_C, D_IN, D_OUT, r_hat)
    if key not in _NC_CACHE:
        _NC_CACHE[key] = _build(*key)
    return _NC_CACHE[key]


def run(inputs, trace=False):
    """Run the SPMD kernel on 8 cores; returns (full_output, results_obj)."""
    from concourse.bass_utils import run_bass_kernel_spmd

    in_maps, r_hat = _prep_in_maps(**inputs)
    nc = _get_nc(r_hat)
    res = run_bass_kernel_spmd(
        nc, in_maps, core_ids=list(range(N_CORES)), trace=trace)
    out = np.concatenate([r["out"] for r in res.results], axis=0)
    return out, res


def kernel(x, W, b, lora_A, lora_B, cu_seqlen):
    out, _ = run(dict(x=x, W=W, b=b, lora_A=lora_A, lora_B=lora_B,
                      cu_seqlen=cu_seqlen))
    return out


# revision 16
# speedup vs baseline: 1.0190x; 1.0190x over previous
"""Trainium2 Bass kernel for nn_BLoraLinear (batched multi-adapter LoRA linear).

Math:  out = x @ W.T + b + sum_s sum_m mask_s(t) * (x @ A[m,s]) @ B[m,s]

Reformulation (exact): with per-(module,segment) adapter columns packed
into Ahat [D_IN, r_hat] / Bhat [r_hat, D_OUT] and a per-token segment
mask MT [r_hat, T],
    out = x @ W.T + b + ((x @ Ahat) * MT.T) @ Bhat
which fuses into one K=(D_IN + r_hat) contraction per output tile:
    out = [x, u] @ [W.T ; Bhat] + b,   u = (x @ Ahat) * MT.T

Sharding: data-parallel over tokens, 1024 tokens per core, zero
collectives.  Since the host knows cu_seqlen values, each core packs
only the adapters of segments overlapping its token range (slots).  Up
to 4 active segments -> r_hat=128 (one contraction chunk); rare draws
with more fall back to a precompiled r_hat=256 variant (always exact).

All matmul operands pre-cast to bf16 on host; f32 accumulation in PSUM;
bias is added during PSUM eviction on the vector engine.
"""

import numpy as np
import ml_dtypes

# Problem shape (hardcoded per spec nn_BLoraLinear_46471546143180).
T, D_IN, D_OUT, R, M, S = 8192, 4096, 4096, 16, 2, 8
N_CORES = 8
T_C = T // N_CORES
MR = M * R                    # adapter columns per segment (32)

BF16 = ml_dtypes.bfloat16


def _build(t_c, d_in, d_out, r_hat):
    """Per-core Bass/Tile program (same NEFF on all cores).

    DRAM layouts are host-prearranged so every DMA is contiguous per
    partition:
      xt   [128, KX, t_c]       xt[p,a,t]    = x[tok0+t, a*128+p]      bf16
      wt   [NB, 128, KX, 512]   wt[n,p,a,c]  = W.T[a*128+p, n*512+c]   bf16
      bh   [128, RC, NB, 512]   bh[p,r,n,c]  = Bhat[r*128+p, n*512+c]  bf16
      ah   [128, KX, r_hat]     ah[p,a,j]    = Ahat[a*128+p, j]        bf16
      mt   [128, RC, t_c]       mt[p,r,t]    = MT[r*128+p, tok0+t]     bf16
      brep [128, d_out]         bias replicated across partitions      bf16
      out  [t_c, d_out]                                                f32
    """
    import concourse.bacc as bacc
    import concourse.mybir as mybir
    from concourse.tile import TileContext

    dt = mybir.dt
    KX = d_in // 128
    RC = r_hat // 128
    NB = d_out // 512
    MB = t_c // 128
    TB = t_c // 512

    nc = bacc.Bacc("TRN2", target_bir_lowering=False)

    xt = nc.dram_tensor("xt", [128, KX, t_c], dt.bfloat16, kind="ExternalInput")
    wt = nc.dram_tensor("wt", [NB, 128, KX, 512], dt.bfloat16, kind="ExternalInput")
    bh = nc.dram_tensor("bh", [128, RC, NB, 512], dt.bfloat16, kind="ExternalInput")
    ah = nc.dram_tensor("ah", [128, KX, r_hat], dt.bfloat16, kind="ExternalInput")
    mt = nc.dram_tensor("mt", [128, RC, t_c], dt.bfloat16, kind="ExternalInput")
    brep = nc.dram_tensor("brep", [128, d_out], dt.bfloat16, kind="ExternalInput")
    out = nc.dram_tensor("out", [t_c, d_out], dt.float32, kind="ExternalOutput")
    warm = nc.dram_tensor("warm", [128, 512], dt.float32, kind="ExternalOutput")

    with TileContext(nc) as tc:
        with tc.tile_pool(name="resident", bufs=1) as res_pool, \
             tc.tile_pool(name="wpool", bufs=2) as w_pool, \
             tc.tile_pool(name="ps", bufs=8, space="PSUM") as ps_pool, \
             tc.tile_pool(name="opool", bufs=4) as o_pool:
            xt_sb = res_pool.tile([128, KX, t_c], dt.bfloat16, name="xt_sb")
            ah_sb = res_pool.tile([128, KX, r_hat], dt.bfloat16, name="ah_sb")
            bh_sb = res_pool.tile([128, RC, NB, 512], dt.bfloat16, name="bh_sb")
            mt_sb = res_pool.tile([128, RC, t_c], dt.bfloat16, name="mt_sb")
            ut_sb = res_pool.tile([128, RC, t_c], dt.bfloat16, name="ut_sb")
            brep_sb = res_pool.tile([128, d_out], dt.bfloat16, name="brep_sb")

            # HAM warmup: PE sits idle ~12us waiting for input DMAs, which
            # leaves the clock gate at 1.2 GHz for the first ~3.4us of real
            # matmuls.  Burn that idle time on dummy matmuls instead so the
            # real stream starts at full clock.  Chained to a scratch
            # output so the instructions aren't dead.
            warm_sb = res_pool.tile([128, 512], dt.bfloat16, name="warm_sb")
            nc.vector.memset(warm_sb[:], 0.0)
            warm_ps = ps_pool.tile([128, 512], dt.float32, name="warm_ps",
                                   tag="ps")
            NWARM = 32
            for i in range(NWARM):
                nc.tensor.matmul(
                    warm_ps[:], warm_sb[:, 0:128], warm_sb[:],
                    start=(i == 0), stop=(i == NWARM - 1))
            warm_o = res_pool.tile([128, 512], dt.float32, name="warm_o")
            nc.vector.tensor_copy(out=warm_o[:], in_=warm_ps[:])
            nc.sync.dma_start(out=warm[:], in_=warm_o[:])

            # Load order tracks first use: ah + the first token-half of x
            # (phase A tb=0 and phase B m<MB/2 only touch that half), then
            # the mask, then W.T block 0 in k-chunk pieces, then the rest.
            step = max(1, KX // 8)
            for a0 in range(0, KX, step):
                a1 = min(a0 + step, KX)
                nc.sync.dma_start(out=ah_sb[:, a0:a1, :], in_=ah[:, a0:a1, :])
                nc.sync.dma_start(out=xt_sb[:, a0:a1, :], in_=xt[:, a0:a1, :])
            nc.sync.dma_start(out=mt_sb[:], in_=mt[:])

            # prefetch W.T column-block n=0 in k-chunk pieces so phase-B
            # matmuls can begin before the whole 4 MB block lands
            wn_tiles = {}

            def load_wn(n):
                t = w_pool.tile([128, KX, 512], dt.bfloat16, name="wn", tag="wn")
                for a0 in range(0, KX, step):
                    a1 = min(a0 + step, KX)
                    nc.sync.dma_start(out=t[:, a0:a1, :], in_=wt[n, :, a0:a1, :])
                wn_tiles[n] = t

            load_wn(0)
            nc.sync.dma_start(out=bh_sb[:], in_=bh[:])
            nc.sync.dma_start(out=brep_sb[:], in_=brep[:])

            # Phase A: uT[j, t] = mask[j, t] * sum_k Ahat[k, j] * xT[k, t]
            for tb in range(TB):
                for rc in range(RC):
                    ps_u = ps_pool.tile([128, 512], dt.float32, name="ps_u", tag="ps")
                    for k in range(KX):
                        nc.tensor.matmul(
                            ps_u[:],
                            ah_sb[:, k, rc * 128:(rc + 1) * 128],
                            xt_sb[:, k, tb * 512:(tb + 1) * 512],
                            start=(k == 0),
                            stop=(k == KX - 1),
                        )
                    nc.vector.tensor_mul(
                        out=ut_sb[:, rc, tb * 512:(tb + 1) * 512],
                        in0=ps_u[:],
                        in1=mt_sb[:, rc, tb * 512:(tb + 1) * 512],
                    )

            # Phase B: out[t, d] = b[d] + sum_k xT/uT[k, t] * [W.T;Bhat][k, d]
            for n in range(NB):
                if n + 1 < NB:
                    load_wn(n + 1)
                wn = wn_tiles.pop(n)
                for m in range(MB):
                    ps_o = ps_pool.tile([128, 512], dt.float32, name="ps_o", tag="ps")
                    for k in range(KX):
                        nc.tensor.matmul(
                            ps_o[:],
                            xt_sb[:, k, m * 128:(m + 1) * 128],
                            wn[:, k, :],
                            start=(k == 0), stop=False,
                        )
                    for r in range(RC):
                        nc.tensor.matmul(
                            ps_o[:],
                            ut_sb[:, r, m * 128:(m + 1) * 128],
                            bh_sb[:, r, n, :],
                            start=False, stop=(r == RC - 1),
                        )
                    o_sb = o_pool.tile([128, 512], dt.float32, name="o_sb")
                    nc.vector.tensor_add(
                        out=o_sb[:], in0=ps_o[:],
                        in1=brep_sb[:, n * 512:(n + 1) * 512])
                    nc.sync.dma_start(
                        out=out[m * 128:(m + 1) * 128, n * 512:(n + 1) * 512],
                        in_=o_sb[:],
                    )

    nc.compile()
    nc.finalize()
    return nc


def _core_slots(cu, t_c, n_cores, n_slots):
    """Per-core list of segments overlapping the core's token range,
    padded with -1 to n_slots.  Returns None if any core needs more."""
    out = []
    for c in range(n_cores):
        lo, hi = c * t_c, (c + 1) * t_c
        slots = [s for s in range(S) if cu[s] < hi and cu[s + 1] > lo
                 and cu[s + 1] > cu[s]]
        if len(slots) > n_slots:
            return None
        out.append(slots + [-1] * (n_slots - len(slots)))
    return out


def _prep_in_maps(x, W, b, lora_A, lora_B, cu_seqlen):
    x = np.asarray(x, dtype=np.float32)
    W = np.asarray(W, dtype=np.float32)
    b = np.asarray(b, dtype=np.float32)
    lora_A = np.asarray(lora_A, dtype=np.float32)
    lora_B = np.asarray(lora_B, dtype=np.float32)
    cu = np.asarray(cu_seqlen).astype(np.int64)

    # full Ahat[k, j], Bhat[j, d], j = (s*M + m)*R + r
    Ahat = np.transpose(lora_A, (2, 1, 0, 3)).reshape(D_IN, S * MR).astype(BF16)
    Bhat = np.transpose(lora_B, (1, 0, 2, 3)).reshape(S * MR, D_OUT).astype(BF16)

    r_hat = 128
    slots = _core_slots(cu, T_C, N_CORES, r_hat // MR)
    if slots is None:
        r_hat = S * MR                                   # 256 fallback
        slots = [list(range(S)) for _ in range(N_CORES)]

    KX = D_IN // 128
    RC = r_hat // 128
    NB = D_OUT // 512

    wt_host = np.ascontiguousarray(
        W.T.astype(BF16).reshape(KX, 128, NB, 512).transpose(2, 1, 0, 3))
    brep_host = np.ascontiguousarray(
        np.broadcast_to(b.astype(BF16), (128, D_OUT)))

    xT = x.astype(BF16).T                                # [D_IN, T] view
    tok = np.arange(T_C)
    in_maps = []
    for c in range(N_CORES):
        sl = slice(c * T_C, (c + 1) * T_C)
        xt_host = np.ascontiguousarray(
            xT[:, sl].reshape(KX, 128, T_C).transpose(1, 0, 2))

        Ah_c = np.zeros((D_IN, r_hat), dtype=BF16)
        Bh_c = np.zeros((r_hat, D_OUT), dtype=BF16)
        MT_c = np.zeros((r_hat, T_C), dtype=BF16)
        for a, s in enumerate(slots[c]):
            if s < 0:
                continue
            Ah_c[:, a * MR:(a + 1) * MR] = Ahat[:, s * MR:(s + 1) * MR]
            Bh_c[a * MR:(a + 1) * MR, :] = Bhat[s * MR:(s + 1) * MR, :]
            lo = max(int(cu[s]) - c * T_C, 0)
            hi = min(int(cu[s + 1]) - c * T_C, T_C)
            if hi > lo:
                MT_c[a * MR:(a + 1) * MR, lo:hi] = 1.0

        ah_host = np.ascontiguousarray(
            Ah_c.reshape(KX, 128, r_hat).transpose(1, 0, 2))
        bh_host = np.ascontiguousarray(
            Bh_c.reshape(RC, 128, NB, 512).transpose(1, 0, 2, 3))
        mt_host = np.ascontiguousarray(
            MT_c.reshape(RC, 128, T_C).transpose(1, 0, 2))
        in_maps.append({
            "xt": xt_host, "wt": wt_host, "bh": bh_host, "ah": ah_host,
            "mt": mt_host, "brep": brep_host,
        })
    return in_maps, r_hat


_NC_CACHE = {}


def _get_nc(r_hat):
    key = (T_C, D_IN, D_OUT, r_hat)
    if key not in _NC_CACHE:
        _NC_CACHE[key] = _build(*key)
    return _NC_CACHE[key]


def run(inputs, trace=False):
    """Run the SPMD kernel on 8 cores; returns (full_output, results_obj)."""
    from concourse.bass_utils import run_bass_kernel_spmd

    in_maps, r_hat = _prep_in_maps(**inputs)
    nc = _get_nc(r_hat)
    res = run_bass_kernel_spmd(
        nc, in_maps, core_ids=list(range(N_CORES)), trace=trace)
    out = np.concatenate([r["out"] for r in res.results], axis=0)
    return out, res


def kernel(x, W, b, lora_A, lora_B, cu_seqlen):
    out, _ = run(dict(x=x, W=W, b=b, lora_A=lora_A, lora_B=lora_B,
                      cu_seqlen=cu_seqlen))
    return out


# revision 21
# speedup vs baseline: 1.0325x; 1.0132x over previous
"""Trainium2 Bass kernel for nn_BLoraLinear (batched multi-adapter LoRA linear).

Math:  out = x @ W.T + b + sum_s sum_m mask_s(t) * (x @ A[m,s]) @ B[m,s]

Reformulation (exact): with per-(module,segment) adapter columns packed
into Ahat [D_IN, r_hat] / Bhat [r_hat, D_OUT] and a per-token segment
mask MT [r_hat, T],
    out = x @ W.T + b + ((x @ Ahat) * MT.T) @ Bhat
which fuses into one K=(D_IN + r_hat) contraction per output tile:
    out = [x, u] @ [W.T ; Bhat] + b,   u = (x @ Ahat) * MT.T

Sharding: data-parallel over tokens, 1024 tokens per core, zero
collectives.  Since the host knows cu_seqlen values, each core packs
only the adapters of segments overlapping its token range (slots).  Up
to 4 active segments -> r_hat=128 (one contraction chunk); rare draws
with more fall back to a precompiled r_hat=256 variant (always exact).

All matmul operands pre-cast to bf16 on host; f32 accumulation in PSUM;
bias is added during PSUM eviction on the vector engine.
"""

import numpy as np
import ml_dtypes

# Problem shape (hardcoded per spec nn_BLoraLinear_46471546143180).
T, D_IN, D_OUT, R, M, S = 8192, 4096, 4096, 16, 2, 8
N_CORES = 8
T_C = T // N_CORES
MR = M * R                    # adapter columns per segment (32)

BF16 = ml_dtypes.bfloat16


def _build(t_c, d_in, d_out, r_hat):
    """Per-core Bass/Tile program (same NEFF on all cores).

    DRAM layouts are host-prearranged so every DMA is contiguous per
    partition:
      xt   [128, KX, t_c]       xt[p,a,t]    = x[tok0+t, a*128+p]      bf16
      wt   [NB, 128, KX, 512]   wt[n,p,a,c]  = W.T[a*128+p, n*512+c]   bf16
      bh   [128, RC, NB, 512]   bh[p,r,n,c]  = Bhat[r*128+p, n*512+c]  bf16
      ah   [128, KX, r_hat]     ah[p,a,j]    = Ahat[a*128+p, j]        bf16
      mt   [128, RC, t_c]       mt[p,r,t]    = MT[r*128+p, tok0+t]     bf16
      brep [128, d_out]         bias replicated across partitions      bf16
      out  [t_c, d_out]                                                f32
    """
    import concourse.bacc as bacc
    import concourse.mybir as mybir
    from concourse.tile import TileContext

    dt = mybir.dt
    KX = d_in // 128
    RC = r_hat // 128
    NB = d_out // 512
    MB = t_c // 128
    TB = t_c // 512

    nc = bacc.Bacc("TRN2", target_bir_lowering=False)

    xt = nc.dram_tensor("xt", [128, KX, t_c], dt.bfloat16, kind="ExternalInput")
    wt = nc.dram_tensor("wt", [NB, 128, KX, 512], dt.bfloat16, kind="ExternalInput")
    bh = nc.dram_tensor("bh", [128, RC, NB, 512], dt.bfloat16, kind="ExternalInput")
    ah = nc.dram_tensor("ah", [128, KX, r_hat], dt.bfloat16, kind="ExternalInput")
    mt = nc.dram_tensor("mt", [128, RC, t_c], dt.bfloat16, kind="ExternalInput")
    brep = nc.dram_tensor("brep", [128, d_out], dt.bfloat16, kind="ExternalInput")
    out = nc.dram_tensor("out", [t_c, d_out], dt.float32, kind="ExternalOutput")

    with TileContext(nc) as tc:
        with tc.tile_pool(name="resident", bufs=1) as res_pool, \
             tc.tile_pool(name="wpool", bufs=2) as w_pool, \
             tc.tile_pool(name="ps", bufs=8, space="PSUM") as ps_pool, \
             tc.tile_pool(name="opool", bufs=4) as o_pool:
            xt_sb = res_pool.tile([128, KX, t_c], dt.bfloat16, name="xt_sb")
            ah_sb = res_pool.tile([128, KX, r_hat], dt.bfloat16, name="ah_sb")
            bh_sb = res_pool.tile([128, RC, NB, 512], dt.bfloat16, name="bh_sb")
            mt_sb = res_pool.tile([128, RC, t_c], dt.bfloat16, name="mt_sb")
            ut_sb = res_pool.tile([128, RC, t_c], dt.bfloat16, name="ut_sb")
            brep_sb = res_pool.tile([128, d_out], dt.bfloat16, name="brep_sb")

            # Load order tracks first use: ah/x chunks interleaved in
            # phase-A consumption order, then the mask, then W.T block 0.
            step = max(1, KX // 8)
            for a0 in range(0, KX, step):
                a1 = min(a0 + step, KX)
                nc.sync.dma_start(out=ah_sb[:, a0:a1, :], in_=ah[:, a0:a1, :])
                nc.sync.dma_start(out=xt_sb[:, a0:a1, :], in_=xt[:, a0:a1, :])
            nc.sync.dma_start(out=mt_sb[:], in_=mt[:])

            # prefetch W.T column-block n=0 in k-chunk pieces so phase-B
            # matmuls can begin before the whole 4 MB block lands
            wn_tiles = {}

            def load_wn(n):
                t = w_pool.tile([128, KX, 512], dt.bfloat16, name="wn", tag="wn")
                for a0 in range(0, KX, step):
                    a1 = min(a0 + step, KX)
                    nc.sync.dma_start(out=t[:, a0:a1, :], in_=wt[n, :, a0:a1, :])
                wn_tiles[n] = t

            load_wn(0)
            nc.sync.dma_start(out=bh_sb[:], in_=bh[:])
            nc.sync.dma_start(out=brep_sb[:], in_=brep[:])

            # Phase A: uT[j, t] = mask[j, t] * sum_k Ahat[k, j] * xT[k, t]
            for tb in range(TB):
                for rc in range(RC):
                    ps_u = ps_pool.tile([128, 512], dt.float32, name="ps_u", tag="ps")
                    for k in range(KX):
                        nc.tensor.matmul(
                            ps_u[:],
                            ah_sb[:, k, rc * 128:(rc + 1) * 128],
                            xt_sb[:, k, tb * 512:(tb + 1) * 512],
                            start=(k == 0),
                            stop=(k == KX - 1),
                        )
                    nc.vector.tensor_mul(
                        out=ut_sb[:, rc, tb * 512:(tb + 1) * 512],
                        in0=ps_u[:],
                        in1=mt_sb[:, rc, tb * 512:(tb + 1) * 512],
                    )

            # Phase B: out[t, d] = b[d] + sum_k xT/uT[k, t] * [W.T;Bhat][k, d]
            for n in range(NB):
                if n + 1 < NB:
                    load_wn(n + 1)
                wn = wn_tiles.pop(n)
                for m in range(MB):
                    ps_o = ps_pool.tile([128, 512], dt.float32, name="ps_o", tag="ps")
                    for k in range(KX):
                        nc.tensor.matmul(
                            ps_o[:],
                            xt_sb[:, k, m * 128:(m + 1) * 128],
                            wn[:, k, :],
                            start=(k == 0), stop=False,
                        )
                    for r in range(RC):
                        nc.tensor.matmul(
                            ps_o[:],
                            ut_sb[:, r, m * 128:(m + 1) * 128],
                            bh_sb[:, r, n, :],
                            start=False, stop=(r == RC - 1),
                        )
                    o_sb = o_pool.tile([128, 512], dt.float32, name="o_sb")
                    nc.vector.tensor_add(
                        out=o_sb[:], in0=ps_o[:],
                        in1=brep_sb[:, n * 512:(n + 1) * 512])
                    nc.sync.dma_start(
                        out=out[m * 128:(m + 1) * 128, n * 512:(n + 1) * 512],
                        in_=o_sb[:],
                    )

    nc.compile()
    nc.finalize()
    return nc


def _core_slots(cu, t_c, n_cores, n_slots):
    """Per-core list of segments overlapping the core's token range,
    padded with -1 to n_slots.  Returns None if any core needs more."""
    out = []
    for c in range(n_cores):
        lo, hi = c * t_c, (c + 1) * t_c
        slots = [s for s in range(S) if cu[s] < hi and cu[s + 1] > lo
                 and cu[s + 1] > cu[s]]
        if len(slots) > n_slots:
            return None
        out.append(slots + [-1] * (n_slots - len(slots)))
    return out


def _prep_in_maps(x, W, b, lora_A, lora_B, cu_seqlen):
    x = np.asarray(x, dtype=np.float32)
    W = np.asarray(W, dtype=np.float32)
    b = np.asarray(b, dtype=np.float32)
    lora_A = np.asarray(lora_A, dtype=np.float32)
    lora_B = np.asarray(lora_B, dtype=np.float32)
    cu = np.asarray(cu_seqlen).astype(np.int64)

    # full Ahat[k, j], Bhat[j, d], j = (s*M + m)*R + r
    Ahat = np.transpose(lora_A, (2, 1, 0, 3)).reshape(D_IN, S * MR).astype(BF16)
    Bhat = np.transpose(lora_B, (1, 0, 2, 3)).reshape(S * MR, D_OUT).astype(BF16)

    r_hat = 128
    slots = _core_slots(cu, T_C, N_CORES, r_hat // MR)
    if slots is None:
        r_hat = S * MR                                   # 256 fallback
        slots = [list(range(S)) for _ in range(N_CORES)]

    KX = D_IN // 128
    RC = r_hat // 128
    NB = D_OUT // 512

    wt_host = np.ascontiguousarray(
        W.T.astype(BF16).reshape(KX, 128, NB, 512).transpose(2, 1, 0, 3))
    brep_host = np.ascontiguousarray(
        np.broadcast_to(b.astype(BF16), (128, D_OUT)))

    xT = x.astype(BF16).T                                # [D_IN, T] view
    tok = np.arange(T_C)
    in_maps = []
    for c in range(N_CORES):
        sl = slice(c * T_C, (c + 1) * T_C)
        xt_host = np.ascontiguousarray(
            xT[:, sl].reshape(KX, 128, T_C).transpose(1, 0, 2))

        Ah_c = np.zeros((D_IN, r_hat), dtype=BF16)
        Bh_c = np.zeros((r_hat, D_OUT), dtype=BF16)
        MT_c = np.zeros((r_hat, T_C), dtype=BF16)
        for a, s in enumerate(slots[c]):
            if s < 0:
                continue
            Ah_c[:, a * MR:(a + 1) * MR] = Ahat[:, s * MR:(s + 1) * MR]
            Bh_c[a * MR:(a + 1) * MR, :] = Bhat[s * MR:(s + 1) * MR, :]
            lo = max(int(cu[s]) - c * T_C, 0)
            hi = min(int(cu[s + 1]) - c * T_C, T_C)
            if hi > lo:
                MT_c[a * MR:(a + 1) * MR, lo:hi] = 1.0

        ah_host = np.ascontiguousarray(
            Ah_c.reshape(KX, 128, r_hat).transpose(1, 0, 2))
        bh_host = np.ascontiguousarray(
            Bh_c.reshape(RC, 128, NB, 512).transpose(1, 0, 2, 3))
        mt_host = np.ascontiguousarray(
            MT_c.reshape(RC, 128, T_C).transpose(1, 0, 2))
        in_maps.append({
            "xt": xt_host, "wt": wt_host, "bh": bh_host, "ah": ah_host,
            "mt": mt_host, "brep": brep_host,
        })
    return in_maps, r_hat


_NC_CACHE = {}


def _get_nc(r_hat):
    key = (T_C, D_IN, D_OUT, r_hat)
    if key not in _NC_CACHE:
        _NC_CACHE[key] = _build(*key)
    return _NC_CACHE[key]


def _ensure_axon_hooks():
    """concourse's trace path imports antenv.axon_hooks, which this image
    lacks.  Provide the tiny get/set registry and wire it to the PJRT
    .so's NTFF entry points when available; degrade to a None hook."""
    import sys
    import types
    if "antenv.axon_hooks" in sys.modules:
        return
    try:
        mod = types.ModuleType("antenv.axon_hooks")
        mod._hook = None
        mod.set_axon_ntff_profile_hook = lambda h: setattr(mod, "_hook", h)
        mod.get_axon_ntff_profile_hook = lambda: mod._hook
        sys.modules["antenv.axon_hooks"] = mod
        import antenv
        antenv.axon_hooks = mod
        try:
            from trn_agent_boot.trn_boot import _ntff_profile_via_ctypes
            mod._hook = _ntff_profile_via_ctypes("/opt/axon/libaxon_pjrt.so")
        except Exception:
            pass
    except Exception:
        pass


def run(inputs, trace=False):
    """Run the SPMD kernel on 8 cores; returns (full_output, results_obj)."""
    _ensure_axon_hooks()
    from concourse.bass_utils import run_bass_kernel_spmd

    in_maps, r_hat = _prep_in_maps(**inputs)
    nc = _get_nc(r_hat)
    res = run_bass_kernel_spmd(
        nc, in_maps, core_ids=list(range(N_CORES)), trace=trace)
    out = np.concatenate([r["out"] for r in res.results], axis=0)
    return out, res


def kernel(x, W, b, lora_A, lora_B, cu_seqlen):
    out, _ = run(dict(x=x, W=W, b=b, lora_A=lora_A, lora_B=lora_B,
                      cu_seqlen=cu_seqlen))
    return out


# revision 27
# speedup vs baseline: 1.0332x; 1.0007x over previous
"""Trainium2 Bass kernel for nn_BLoraLinear (batched multi-adapter LoRA linear).

Math:  out = x @ W.T + b + sum_s sum_m mask_s(t) * (x @ A[m,s]) @ B[m,s]

Reformulation (exact): with per-(module,segment) adapter columns packed
into Ahat [D_IN, r_hat] / Bhat [r_hat, D_OUT] and a per-token segment
mask MT [r_hat, T],
    out = x @ W.T + b + ((x @ Ahat) * MT.T) @ Bhat
which fuses into one K=(D_IN + r_hat) contraction per output tile:
    out = [x, u] @ [W.T ; Bhat] + b,   u = (x @ Ahat) * MT.T

Sharding: data-parallel over tokens, 1024 tokens per core, zero
collectives.  Since the host knows cu_seqlen values, each core packs
only the adapters of segments overlapping its token range (slots).  Up
to 4 active segments -> r_hat=128 (one contraction chunk); rare draws
with more fall back to a precompiled r_hat=256 variant (always exact).

All matmul operands pre-cast to bf16 on host; f32 accumulation in PSUM;
bias is added during PSUM eviction on the vector engine.
"""

import numpy as np
import ml_dtypes

# Problem shape (hardcoded per spec nn_BLoraLinear_46471546143180).
T, D_IN, D_OUT, R, M, S = 8192, 4096, 4096, 16, 2, 8
N_CORES = 8
T_C = T // N_CORES
MR = M * R                    # adapter columns per segment (32)

BF16 = ml_dtypes.bfloat16


def _build(t_c, d_in, d_out, r_hat):
    """Per-core Bass/Tile program (same NEFF on all cores).

    DRAM layouts are host-prearranged so every DMA is contiguous per
    partition:
      xt   [128, KX, t_c]       xt[p,a,t]    = x[tok0+t, a*128+p]      bf16
      wt   [NB, 128, KX, 512]   wt[n,p,a,c]  = W.T[a*128+p, n*512+c]   bf16
      bh   [128, RC, NB, 512]   bh[p,r,n,c]  = Bhat[r*128+p, n*512+c]  bf16
      ah   [128, KX, r_hat]     ah[p,a,j]    = Ahat[a*128+p, j]        bf16
      mt   [128, RC, t_c]       mt[p,r,t]    = MT[r*128+p, tok0+t]     bf16
      brep [128, d_out]         bias replicated across partitions      bf16
      out  [t_c, d_out]                                                f32
    """
    import concourse.bacc as bacc
    import concourse.mybir as mybir
    from concourse.tile import TileContext

    dt = mybir.dt
    KX = d_in // 128
    RC = r_hat // 128
    NB = d_out // 512
    MB = t_c // 128
    TB = t_c // 512

    nc = bacc.Bacc("TRN2", target_bir_lowering=False)

    xt = nc.dram_tensor("xt", [128, KX, t_c], dt.bfloat16, kind="ExternalInput")
    wt = nc.dram_tensor("wt", [NB, 128, KX, 512], dt.bfloat16, kind="ExternalInput")
    bh = nc.dram_tensor("bh", [128, RC, NB, 512], dt.bfloat16, kind="ExternalInput")
    ah = nc.dram_tensor("ah", [128, KX, r_hat], dt.bfloat16, kind="ExternalInput")
    mt = nc.dram_tensor("mt", [128, RC, t_c], dt.bfloat16, kind="ExternalInput")
    brep = nc.dram_tensor("brep", [128, d_out], dt.bfloat16, kind="ExternalInput")
    out = nc.dram_tensor("out", [t_c, d_out], dt.float32, kind="ExternalOutput")

    with TileContext(nc) as tc:
        with tc.tile_pool(name="resident", bufs=1) as res_pool, \
             tc.tile_pool(name="wpool", bufs=2) as w_pool, \
             tc.tile_pool(name="ps", bufs=8, space="PSUM") as ps_pool, \
             tc.tile_pool(name="opool", bufs=4) as o_pool:
            xt_sb = res_pool.tile([128, KX, t_c], dt.bfloat16, name="xt_sb")
            ah_sb = res_pool.tile([128, KX, r_hat], dt.bfloat16, name="ah_sb")
            bh_sb = res_pool.tile([128, RC, NB, 512], dt.bfloat16, name="bh_sb")
            mt_sb = res_pool.tile([128, RC, t_c], dt.bfloat16, name="mt_sb")
            ut_sb = res_pool.tile([128, RC, t_c], dt.bfloat16, name="ut_sb")
            brep_sb = res_pool.tile([128, d_out], dt.bfloat16, name="brep_sb")

            # Startup DMA order tracks first use: ah + x token-half 0 feed
            # phase A tb=0; W.T block 0 feeds phase B n=0 m<MB/2 (which only
            # reads ut half 0); x half 1 + phase A tb=1 are deferred until
            # phase B n=0 is already keeping the PE busy.
            step = max(1, KX // 8)
            h0 = min(512, t_c)
            # first chunks arrive as single-k DMAs so the PE can issue its
            # first matmuls the moment its preamble finishes (~7 us)
            for a0 in range(0, KX, step):
                a1 = min(a0 + step, KX)
                nc.sync.dma_start(out=ah_sb[:, a0:a1, :], in_=ah[:, a0:a1, :])
                nc.sync.dma_start(out=xt_sb[:, a0:a1, 0:h0],
                                  in_=xt[:, a0:a1, 0:h0])
            nc.sync.dma_start(out=mt_sb[:], in_=mt[:])

            # prefetch W.T column-block n in k-chunk pieces so phase-B
            # matmuls can begin before the whole 4 MB block lands
            wn_tiles = {}

            def load_wn(n, split=False):
                t = w_pool.tile([128, KX, 512], dt.bfloat16, name="wn", tag="wn")
                if split:
                    for a0 in range(0, KX, step):
                        a1 = min(a0 + step, KX)
                        nc.sync.dma_start(out=t[:, a0:a1, :],
                                          in_=wt[n, :, a0:a1, :])
                else:
                    nc.sync.dma_start(out=t[:], in_=wt[n])
                wn_tiles[n] = t

            load_wn(0, split=True)
            nc.sync.dma_start(out=bh_sb[:], in_=bh[:])
            if h0 < t_c:
                nc.sync.dma_start(out=xt_sb[:, :, h0:], in_=xt[:, :, h0:])
            nc.sync.dma_start(out=brep_sb[:], in_=brep[:])

            # Phase A (one tb block): uT[j,t] = mask[j,t] * sum_k Ahat[k,j]*xT[k,t]
            def phase_a(tb):
                for rc in range(RC):
                    ps_u = ps_pool.tile([128, 512], dt.float32, name="ps_u",
                                        tag="ps")
                    for k in range(KX):
                        nc.tensor.matmul(
                            ps_u[:],
                            ah_sb[:, k, rc * 128:(rc + 1) * 128],
                            xt_sb[:, k, tb * 512:(tb + 1) * 512],
                            start=(k == 0),
                            stop=(k == KX - 1),
                        )
                    nc.vector.tensor_mul(
                        out=ut_sb[:, rc, tb * 512:(tb + 1) * 512],
                        in0=ps_u[:],
                        in1=mt_sb[:, rc, tb * 512:(tb + 1) * 512],
                    )

            # Phase B group: out[t,d] = b[d] + sum_k xT/uT[k,t] * [W.T;Bhat][k,d]
            def phase_b_group(n, m, wn):
                ps_o = ps_pool.tile([128, 512], dt.float32, name="ps_o",
                                    tag="ps")
                for k in range(KX):
                    nc.tensor.matmul(
                        ps_o[:],
                        xt_sb[:, k, m * 128:(m + 1) * 128],
                        wn[:, k, :],
                        start=(k == 0), stop=False,
                    )
                for r in range(RC):
                    nc.tensor.matmul(
                        ps_o[:],
                        ut_sb[:, r, m * 128:(m + 1) * 128],
                        bh_sb[:, r, n, :],
                        start=False, stop=(r == RC - 1),
                    )
                o_sb = o_pool.tile([128, 512], dt.float32, name="o_sb")
                nc.vector.tensor_add(
                    out=o_sb[:], in0=ps_o[:],
                    in1=brep_sb[:, n * 512:(n + 1) * 512])
                nc.sync.dma_start(
                    out=out[m * 128:(m + 1) * 128, n * 512:(n + 1) * 512],
                    in_=o_sb[:],
                )

            phase_a(0)
            mb_half = MB // TB
            for n in range(NB):
                if n + 1 < NB:
                    load_wn(n + 1)
                wn = wn_tiles.pop(n)
                for m in range(mb_half):
                    phase_b_group(n, m, wn)
                if n == 0:
                    for tb in range(1, TB):
                        phase_a(tb)
                for m in range(mb_half, MB):
                    phase_b_group(n, m, wn)

    nc.compile()
    nc.finalize()
    return nc


def _core_slots(cu, t_c, n_cores, n_slots):
    """Per-core list of segments overlapping the core's token range,
    padded with -1 to n_slots.  Returns None if any core needs more."""
    out = []
    for c in range(n_cores):
        lo, hi = c * t_c, (c + 1) * t_c
        slots = [s for s in range(S) if cu[s] < hi and cu[s + 1] > lo
                 and cu[s + 1] > cu[s]]
        if len(slots) > n_slots:
            return None
        out.append(slots + [-1] * (n_slots - len(slots)))
    return out


def _prep_in_maps(x, W, b, lora_A, lora_B, cu_seqlen):
    x = np.asarray(x, dtype=np.float32)
    W = np.asarray(W, dtype=np.float32)
    b = np.asarray(b, dtype=np.float32)
    lora_A = np.asarray(lora_A, dtype=np.float32)
    lora_B = np.asarray(lora_B, dtype=np.float32)
    cu = np.asarray(cu_seqlen).astype(np.int64)

    # full Ahat[k, j], Bhat[j, d], j = (s*M + m)*R + r
    Ahat = np.transpose(lora_A, (2, 1, 0, 3)).reshape(D_IN, S * MR).astype(BF16)
    Bhat = np.transpose(lora_B, (1, 0, 2, 3)).reshape(S * MR, D_OUT).astype(BF16)

    r_hat = 128
    slots = _core_slots(cu, T_C, N_CORES, r_hat // MR)
    if slots is None:
        r_hat = S * MR                                   # 256 fallback
        slots = [list(range(S)) for _ in range(N_CORES)]

    KX = D_IN // 128
    RC = r_hat // 128
    NB = D_OUT // 512

    wt_host = np.ascontiguousarray(
        W.T.astype(BF16).reshape(KX, 128, NB, 512).transpose(2, 1, 0, 3))
    brep_host = np.ascontiguousarray(
        np.broadcast_to(b.astype(BF16), (128, D_OUT)))

    xT = x.astype(BF16).T                                # [D_IN, T] view
    tok = np.arange(T_C)
    in_maps = []
    for c in range(N_CORES):
        sl = slice(c * T_C, (c + 1) * T_C)
        xt_host = np.ascontiguousarray(
            xT[:, sl].reshape(KX, 128, T_C).transpose(1, 0, 2))

        Ah_c = np.zeros((D_IN, r_hat), dtype=BF16)
        Bh_c = np.zeros((r_hat, D_OUT), dtype=BF16)
        MT_c = np.zeros((r_hat, T_C), dtype=BF16)
        for a, s in enumerate(slots[c]):
            if s < 0:
                continue
            Ah_c[:, a * MR:(a + 1) * MR] = Ahat[:, s * MR:(s + 1) * MR]
            Bh_c[a * MR:(a + 1) * MR, :] = Bhat[s * MR:(s + 1) * MR, :]
            lo = max(int(cu[s]) - c * T_C, 0)
            hi = min(int(cu[s + 1]) - c * T_C, T_C)
            if hi > lo:
                MT_c[a * MR:(a + 1) * MR, lo:hi] = 1.0

        ah_host = np.ascontiguousarray(
            Ah_c.reshape(KX, 128, r_hat).transpose(1, 0, 2))
        bh_host = np.ascontiguousarray(
            Bh_c.reshape(RC, 128, NB, 512).transpose(1, 0, 2, 3))
        mt_host = np.ascontiguousarray(
            MT_c.reshape(RC, 128, T_C).transpose(1, 0, 2))
        in_maps.append({
            "xt": xt_host, "wt": wt_host, "bh": bh_host, "ah": ah_host,
            "mt": mt_host, "brep": brep_host,
        })
    return in_maps, r_hat


_NC_CACHE = {}


def _get_nc(r_hat):
    key = (T_C, D_IN, D_OUT, r_hat)
    if key not in _NC_CACHE:
        _NC_CACHE[key] = _build(*key)
    return _NC_CACHE[key]


def _ensure_axon_hooks():
    """concourse's trace path imports antenv.axon_hooks, which this image
    lacks.  Provide the tiny get/set registry and wire it to the PJRT
    .so's NTFF entry points when available; degrade to a None hook."""
    import sys
    import types
    if "antenv.axon_hooks" in sys.modules:
        return
    try:
        mod = types.ModuleType("antenv.axon_hooks")
        mod._hook = None
        mod.set_axon_ntff_profile_hook = lambda h: setattr(mod, "_hook", h)
        mod.get_axon_ntff_profile_hook = lambda: mod._hook
        sys.modules["antenv.axon_hooks"] = mod
        import antenv
        antenv.axon_hooks = mod
        try:
            from trn_agent_boot.trn_boot import _ntff_profile_via_ctypes
            mod._hook = _ntff_profile_via_ctypes("/opt/axon/libaxon_pjrt.so")
        except Exception:
            pass
    except Exception:
        pass


def run(inputs, trace=False):
    """Run the SPMD kernel on 8 cores; returns (full_output, results_obj)."""
    _ensure_axon_hooks()
    from concourse.bass_utils import run_bass_kernel_spmd

    in_maps, r_hat = _prep_in_maps(**inputs)
    nc = _get_nc(r_hat)
    res = run_bass_kernel_spmd(
        nc, in_maps, core_ids=list(range(N_CORES)), trace=trace)
    out = np.concatenate([r["out"] for r in res.results], axis=0)
    return out, res


def kernel(x, W, b, lora_A, lora_B, cu_seqlen):
    out, _ = run(dict(x=x, W=W, b=b, lora_A=lora_A, lora_B=lora_B,
                      cu_seqlen=cu_seqlen))
    return out


# revision 32
# speedup vs baseline: 1.0335x; 1.0003x over previous
"""Trainium2 Bass kernel for nn_BLoraLinear (batched multi-adapter LoRA linear).

Math:  out = x @ W.T + b + sum_s sum_m mask_s(t) * (x @ A[m,s]) @ B[m,s]

Reformulation (exact): with per-(module,segment) adapter columns packed
into Ahat [D_IN, r_hat] / Bhat [r_hat, D_OUT] and a per-token segment
mask MT [r_hat, T],
    out = x @ W.T + b + ((x @ Ahat) * MT.T) @ Bhat
which fuses into one K=(D_IN + r_hat) contraction per output tile:
    out = [x, u] @ [W.T ; Bhat] + b,   u = (x @ Ahat) * MT.T

Sharding: data-parallel over tokens, 1024 tokens per core, zero
collectives.  Since the host knows cu_seqlen values, each core packs
only the adapters of segments overlapping its token range (slots).  Up
to 4 active segments -> r_hat=128 (one contraction chunk); rare draws
with more fall back to a precompiled r_hat=256 variant (always exact).

All matmul operands pre-cast to bf16 on host; f32 accumulation in PSUM;
bias is added during PSUM eviction on the vector engine.
"""

import numpy as np
import ml_dtypes

# Problem shape (hardcoded per spec nn_BLoraLinear_46471546143180).
T, D_IN, D_OUT, R, M, S = 8192, 4096, 4096, 16, 2, 8
N_CORES = 8
T_C = T // N_CORES
MR = M * R                    # adapter columns per segment (32)

BF16 = ml_dtypes.bfloat16


def _build(t_c, d_in, d_out, r_hat):
    """Per-core Bass/Tile program (same NEFF on all cores).

    DRAM layouts are host-prearranged so every DMA is contiguous per
    partition:
      xt   [128, KX, t_c]       xt[p,a,t]    = x[tok0+t, a*128+p]      bf16
      wt   [NB, 128, KX, 512]   wt[n,p,a,c]  = W.T[a*128+p, n*512+c]   bf16
      bh   [128, RC, NB, 512]   bh[p,r,n,c]  = Bhat[r*128+p, n*512+c]  bf16
      ah   [128, KX, r_hat]     ah[p,a,j]    = Ahat[a*128+p, j]        bf16
      mt   [128, RC, t_c]       mt[p,r,t]    = MT[r*128+p, tok0+t]     bf16
      brep [128, d_out]         bias replicated across partitions      bf16
      out  [t_c, d_out]                                                f32
    """
    import concourse.bacc as bacc
    import concourse.mybir as mybir
    from concourse.tile import TileContext

    dt = mybir.dt
    KX = d_in // 128
    RC = r_hat // 128
    NB = d_out // 512
    MB = t_c // 128
    TB = t_c // 512

    nc = bacc.Bacc("TRN2", target_bir_lowering=False)

    xt = nc.dram_tensor("xt", [128, KX, t_c], dt.bfloat16, kind="ExternalInput")
    wt = nc.dram_tensor("wt", [NB, 128, KX, 512], dt.bfloat16, kind="ExternalInput")
    bh = nc.dram_tensor("bh", [128, RC, NB, 512], dt.bfloat16, kind="ExternalInput")
    ah = nc.dram_tensor("ah", [128, KX, r_hat], dt.bfloat16, kind="ExternalInput")
    mt = nc.dram_tensor("mt", [128, RC, t_c], dt.bfloat16, kind="ExternalInput")
    brep = nc.dram_tensor("brep", [128, d_out], dt.bfloat16, kind="ExternalInput")
    out = nc.dram_tensor("out", [t_c, d_out], dt.float32, kind="ExternalOutput")

    with TileContext(nc) as tc:
        with tc.tile_pool(name="resident", bufs=1) as res_pool, \
             tc.tile_pool(name="wpool", bufs=2) as w_pool, \
             tc.tile_pool(name="ps", bufs=8, space="PSUM") as ps_pool, \
             tc.tile_pool(name="opool", bufs=4) as o_pool:
            xt_sb = res_pool.tile([128, KX, t_c], dt.bfloat16, name="xt_sb")
            ah_sb = res_pool.tile([128, KX, r_hat], dt.bfloat16, name="ah_sb")
            bh_sb = res_pool.tile([128, RC, NB, 512], dt.bfloat16, name="bh_sb")
            mt_sb = res_pool.tile([128, RC, t_c], dt.bfloat16, name="mt_sb")
            ut_sb = res_pool.tile([128, RC, t_c], dt.bfloat16, name="ut_sb")
            brep_sb = res_pool.tile([128, d_out], dt.bfloat16, name="brep_sb")

            # Startup DMA order tracks first use: ah + x token-half 0 feed
            # phase A tb=0; W.T block 0 feeds phase B n=0 m<MB/2 (which only
            # reads ut half 0); x half 1 + phase A tb=1 are deferred until
            # phase B n=0 is already keeping the PE busy.
            step = max(1, KX // 8)
            h0 = min(512, t_c)
            # first chunks arrive as single-k DMAs so the PE can issue its
            # first matmuls the moment its preamble finishes (~7 us)
            for a0 in range(0, KX, step):
                a1 = min(a0 + step, KX)
                nc.sync.dma_start(out=ah_sb[:, a0:a1, :], in_=ah[:, a0:a1, :])
                nc.sync.dma_start(out=xt_sb[:, a0:a1, 0:h0],
                                  in_=xt[:, a0:a1, 0:h0])
            nc.sync.dma_start(out=mt_sb[:], in_=mt[:])

            # prefetch W.T column-block n in k-chunk pieces so phase-B
            # matmuls can begin before the whole 4 MB block lands
            wn_tiles = {}

            def load_wn(n, split=0, mid=None):
                t = w_pool.tile([128, KX, 512], dt.bfloat16, name="wn", tag="wn")
                if split:
                    for i, a0 in enumerate(range(0, KX, split)):
                        a1 = min(a0 + split, KX)
                        nc.sync.dma_start(out=t[:, a0:a1, :],
                                          in_=wt[n, :, a0:a1, :])
                        if mid is not None and a1 == KX // 2:
                            mid()
                else:
                    nc.sync.dma_start(out=t[:], in_=wt[n])
                wn_tiles[n] = t

            # bh is first read by group (n=0, m=0)'s final matmul, i.e. after
            # ~half of wn0 has been consumed — land it mid-way through wn0
            load_wn(0, split=max(1, step // 2),
                    mid=lambda: nc.sync.dma_start(out=bh_sb[:], in_=bh[:]))
            if h0 < t_c:
                nc.sync.dma_start(out=xt_sb[:, :, h0:], in_=xt[:, :, h0:])
            nc.sync.dma_start(out=brep_sb[:], in_=brep[:])

            # Phase A (one tb block): uT[j,t] = mask[j,t] * sum_k Ahat[k,j]*xT[k,t]
            def phase_a(tb):
                for rc in range(RC):
                    ps_u = ps_pool.tile([128, 512], dt.float32, name="ps_u",
                                        tag="ps")
                    for k in range(KX):
                        nc.tensor.matmul(
                            ps_u[:],
                            ah_sb[:, k, rc * 128:(rc + 1) * 128],
                            xt_sb[:, k, tb * 512:(tb + 1) * 512],
                            start=(k == 0),
                            stop=(k == KX - 1),
                        )
                    nc.vector.tensor_mul(
                        out=ut_sb[:, rc, tb * 512:(tb + 1) * 512],
                        in0=ps_u[:],
                        in1=mt_sb[:, rc, tb * 512:(tb + 1) * 512],
                    )

            # Phase B group: out[t,d] = b[d] + sum_k xT/uT[k,t] * [W.T;Bhat][k,d]
            def phase_b_group(n, m, wn):
                ps_o = ps_pool.tile([128, 512], dt.float32, name="ps_o",
                                    tag="ps")
                for k in range(KX):
                    nc.tensor.matmul(
                        ps_o[:],
                        xt_sb[:, k, m * 128:(m + 1) * 128],
                        wn[:, k, :],
                        start=(k == 0), stop=False,
                    )
                for r in range(RC):
                    nc.tensor.matmul(
                        ps_o[:],
                        ut_sb[:, r, m * 128:(m + 1) * 128],
                        bh_sb[:, r, n, :],
                        start=False, stop=(r == RC - 1),
                    )
                o_sb = o_pool.tile([128, 512], dt.float32, name="o_sb")
                nc.vector.tensor_add(
                    out=o_sb[:], in0=ps_o[:],
                    in1=brep_sb[:, n * 512:(n + 1) * 512])
                nc.sync.dma_start(
                    out=out[m * 128:(m + 1) * 128, n * 512:(n + 1) * 512],
                    in_=o_sb[:],
                )

            phase_a(0)
            mb_half = MB // TB
            for n in range(NB):
                if n + 1 < NB:
                    load_wn(n + 1)
                wn = wn_tiles.pop(n)
                for m in range(mb_half):
                    phase_b_group(n, m, wn)
                if n == 0:
                    for tb in range(1, TB):
                        phase_a(tb)
                for m in range(mb_half, MB):
                    phase_b_group(n, m, wn)

    nc.compile()
    nc.finalize()
    return nc


def _core_slots(cu, t_c, n_cores, n_slots):
    """Per-core list of segments overlapping the core's token range,
    padded with -1 to n_slots.  Returns None if any core needs more."""
    out = []
    for c in range(n_cores):
        lo, hi = c * t_c, (c + 1) * t_c
        slots = [s for s in range(S) if cu[s] < hi and cu[s + 1] > lo
                 and cu[s + 1] > cu[s]]
        if len(slots) > n_slots:
            return None
        out.append(slots + [-1] * (n_slots - len(slots)))
    return out


def _prep_in_maps(x, W, b, lora_A, lora_B, cu_seqlen):
    x = np.asarray(x, dtype=np.float32)
    W = np.asarray(W, dtype=np.float32)
    b = np.asarray(b, dtype=np.float32)
    lora_A = np.asarray(lora_A, dtype=np.float32)
    lora_B = np.asarray(lora_B, dtype=np.float32)
    cu = np.asarray(cu_seqlen).astype(np.int64)

    # full Ahat[k, j], Bhat[j, d], j = (s*M + m)*R + r
    Ahat = np.transpose(lora_A, (2, 1, 0, 3)).reshape(D_IN, S * MR).astype(BF16)
    Bhat = np.transpose(lora_B, (1, 0, 2, 3)).reshape(S * MR, D_OUT).astype(BF16)

    r_hat = 128
    slots = _core_slots(cu, T_C, N_CORES, r_hat // MR)
    if slots is None:
        r_hat = S * MR                                   # 256 fallback
        slots = [list(range(S)) for _ in range(N_CORES)]

    KX = D_IN // 128
    RC = r_hat // 128
    NB = D_OUT // 512

    wt_host = np.ascontiguousarray(
        W.T.astype(BF16).reshape(KX, 128, NB, 512).transpose(2, 1, 0, 3))
    brep_host = np.ascontiguousarray(
        np.broadcast_to(b.astype(BF16), (128, D_OUT)))

    xT = x.astype(BF16).T                                # [D_IN, T] view
    tok = np.arange(T_C)
    in_maps = []
    for c in range(N_CORES):
        sl = slice(c * T_C, (c + 1) * T_C)
        xt_host = np.ascontiguousarray(
            xT[:, sl].reshape(KX, 128, T_C).transpose(1, 0, 2))

        Ah_c = np.zeros((D_IN, r_hat), dtype=BF16)
        Bh_c = np.zeros((r_hat, D_OUT), dtype=BF16)
        MT_c = np.zeros((r_hat, T_C), dtype=BF16)
        for a, s in enumerate(slots[c]):
            if s < 0:
                continue
            Ah_c[:, a * MR:(a + 1) * MR] = Ahat[:, s * MR:(s + 1) * MR]
            Bh_c[a * MR:(a + 1) * MR, :] = Bhat[s * MR:(s + 1) * MR, :]
            lo = max(int(cu[s]) - c * T_C, 0)
            hi = min(int(cu[s + 1]) - c * T_C, T_C)
            if hi > lo:
                MT_c[a * MR:(a + 1) * MR, lo:hi] = 1.0

        ah_host = np.ascontiguousarray(
            Ah_c.reshape(KX, 128, r_hat).transpose(1, 0, 2))
        bh_host = np.ascontiguousarray(
            Bh_c.reshape(RC, 128, NB, 512).transpose(1, 0, 2, 3))
        mt_host = np.ascontiguousarray(
            MT_c.reshape(RC, 128, T_C).transpose(1, 0, 2))
        in_maps.append({
            "xt": xt_host, "wt": wt_host, "bh": bh_host, "ah": ah_host,
            "mt": mt_host, "brep": brep_host,
        })
    return in_maps, r_hat


_NC_CACHE = {}


def _get_nc(r_hat):
    key = (T_C, D_IN, D_OUT, r_hat)
    if key not in _NC_CACHE:
        _NC_CACHE[key] = _build(*key)
    return _NC_CACHE[key]


def _ensure_axon_hooks():
    """concourse's trace path imports antenv.axon_hooks, which this image
    lacks.  Provide the tiny get/set registry and wire it to the PJRT
    .so's NTFF entry points when available; degrade to a None hook."""
    import sys
    import types
    if "antenv.axon_hooks" in sys.modules:
        return
    try:
        mod = types.ModuleType("antenv.axon_hooks")
        mod._hook = None
        mod.set_axon_ntff_profile_hook = lambda h: setattr(mod, "_hook", h)
        mod.get_axon_ntff_profile_hook = lambda: mod._hook
        sys.modules["antenv.axon_hooks"] = mod
        import antenv
        antenv.axon_hooks = mod
        try:
            from trn_agent_boot.trn_boot import _ntff_profile_via_ctypes
            mod._hook = _ntff_profile_via_ctypes("/opt/axon/libaxon_pjrt.so")
        except Exception:
            pass
    except Exception:
        pass


def run(inputs, trace=False):
    """Run the SPMD kernel on 8 cores; returns (full_output, results_obj)."""
    _ensure_axon_hooks()
    from concourse.bass_utils import run_bass_kernel_spmd

    in_maps, r_hat = _prep_in_maps(**inputs)
    nc = _get_nc(r_hat)
    res = run_bass_kernel_spmd(
        nc, in_maps, core_ids=list(range(N_CORES)), trace=trace)
    out = np.concatenate([r["out"] for r in res.results], axis=0)
    return out, res


def kernel(x, W, b, lora_A, lora_B, cu_seqlen):
    out, _ = run(dict(x=x, W=W, b=b, lora_A=lora_A, lora_B=lora_B,
                      cu_seqlen=cu_seqlen))
    return out
